# revision 5
# baseline (speedup 1.0000x reference)
"""C3DLoss kernel for Trainium2 — 8-core batch-parallel, raw-Bass implementation.

Per core = one batch frame b (tgt pairing partner tb = b^1):
    partial = sum over both terms (same-frame, cross-frame), all 25 shifts
              delta in [-2,2]^2, all pixels p of
        mref(p) * mq(p+delta) * exp(-50*(|xyz_r(p)-xyz_q(p+d)|^2
                                         + |rgb_r(p)-rgb_q(p+d)|^2))
    loss = -(sum of partials) / max(sum(depth_gt_mask), 1)

v4 design notes (measured-HW-behavior driven):
  - Two-input DVE tensor_tensor runs at 1 elem/cycle/lane regardless of
    layout (the 2x_1p fast mode only engages for single-stream ops), so
    subtracts use COMPACT 2-D access patterns (1216 useful cols, no halo
    waste).  Squares read one stream (mult(d,d)) as a contiguous 1-D AP
    and do hit 2x.  GpSimd wants 2-D patterns (its 8 Q7 cores
    parallelize across rows; a long 1-D run serializes 2.3x slower).
  - fp16 planes throughout; masks folded into feature channel 3
    (ra3=+20*(1-mref), qa3=-20*(1-mq)) so one selA matmul pass covers
    xyz+mask and one selB pass covers rgb.
  - Per-slot split: DVE does subA always, sqA always, subB on every 8th
    slot; GpSimd does the other 7/8 of subB; ScalarE squares rgb diffs
    in slot-pairs and runs exp+accumulate from PSUM.
  - PE per 4-slot batch: all selA matmuls, then all selB (weight reuse),
    compact contiguous rhs chunks, 8 PSUM banks, then_inc signaling.
"""

import sys

for _p in ("/opt/trn_rl_repo", "/opt/pypackages"):
    if _p not in sys.path:
        sys.path.insert(0, _p)

from contextlib import ExitStack

import numpy as np

import concourse.bass as bass
import concourse.mybir as mybir
from concourse.ap import AP
from concourse.alu_op_type import AluOpType

F32 = mybir.dt.float32
F16 = mybir.dt.float16

R = 2
G = 32            # W-blocks; one shift-slot = 32 PSUM partitions
CA = 4            # tile A channels: x, y, z, mask
CB = 3            # tile B channels: r, g, b
SBATCH = 4        # shift slots per 128-partition PSUM bank
NPSUM = 8         # rotating PSUM banks (unit = (batch, chunk))
NSQA = 8          # rotating sqa buffers (per slot)
NDP = 4           # rotating db pair buffers (per 2 slots)
NSQB = 4          # rotating sqb pair buffers (per 2 slots)
DVE_B_EVERY = 8   # DVE handles subB on slots J % DVE_B_EVERY == 0
MK = 20.0         # mask channel scale; (2*MK)^2 = 1600 >> 1/50
EXP_SCALE = -50.0


class Cfg:
    def __init__(self, H=352, W=1216, HS=32):
        assert W % G == 0 and H % HS == 0
        self.H, self.W, self.HS = H, W, HS
        self.WB = W // G                      # 38
        self.WBH = self.WB + 2 * R            # 42
        self.Hp = H + 2 * R                   # 356
        self.NSLAB = H // HS                  # 11
        self.NQ = G * self.Hp * self.WBH      # haloed plane elems
        self.QF = (HS + 2 * R) * self.WBH     # query tile free size 1512
        self.RF = HS * self.WBH               # ref tile free size 1344
        self.SF = HS * self.WB                # compact slot size 1216
        # col-chunks of the compact slot (PSUM bank <= 512 f32/partition)
        cw = (512 // self.WB) * self.WB       # 494
        self.chunks = []
        o = 0
        while o < self.SF:
            self.chunks.append((o, min(cw, self.SF - o)))
            o += cw
        self.NC = len(self.chunks)            # 3
        self.slots = [(t, dy, dx) for t in (0, 1)
                      for dy in range(-R, R + 1) for dx in range(-R, R + 1)]
        self.NS = len(self.slots)             # 50
        assert self.NS % 2 == 0
        self.batches = [self.slots[i:i + SBATCH]
                        for i in range(0, self.NS, SBATCH)]
        self.NB = len(self.batches)           # 13
        self.TOTS = self.NSLAB * self.NS      # 550 slots
        self.TOTB = self.NSLAB * self.NB      # 143 batches
        self.n_acc = self.TOTB * self.NC      # 429 acc columns
        # per-slot subB producer bookkeeping
        self.b_dve = [J % DVE_B_EVERY == 0 for J in range(self.TOTS)]
        # cumulative producer instruction counts AFTER slot J's subB
        self.cnt_v = []   # DVE subB count
        self.cnt_p = []   # Pool subB count
        cv = cp = 0
        for J in range(self.TOTS):
            if self.b_dve[J]:
                cv += 1
            else:
                cp += 1
            self.cnt_v.append(cv)
            self.cnt_p.append(cp)

    def slot_batch(self, J):
        return (J // self.NS) * self.NB + (J % self.NS) // SBATCH


def _apv(t_ap, p0, pcnt, free_dims, free_off=0):
    pstride = t_ap.ap[0][0]
    base = t_ap.offset + p0 * pstride + free_off
    return AP(t_ap.tensor, base, [[pstride, pcnt]] + [list(d) for d in free_dims])


def _dram_ap(handle, offset, dims):
    a = handle[:]
    return AP(a.tensor, a.offset + offset, [list(d) for d in dims])


def make_selA():
    s = np.zeros((CA * G, G), dtype=np.float16)
    for c in range(CA):
        for g in range(G):
            s[c * G + g, g] = 1
    return s


def make_selB():
    s = np.zeros((CB * G, G), dtype=np.float16)
    for c in range(CB):
        for g in range(G):
            s[c * G + g, g] = 1
    return s


def emit(nc: bass.Bass, cfg: Cfg):
    HS, WB, WBH, Hp = cfg.HS, cfg.WB, cfg.WBH, cfg.Hp
    NQ, QF, RF, SF = cfg.NQ, cfg.QF, cfg.RF, cfg.SF
    NSLAB, NB, NC, NS = cfg.NSLAB, cfg.NB, cfg.NC, cfg.NS
    Act = mybir.ActivationFunctionType
    HpW = Hp * WBH

    dp = nc.declare_dram_parameter
    qa_d = dp("qa_d", [2, CA, NQ], F16, isOutput=False)
    ra_d = dp("ra_d", [2, CA, NQ], F16, isOutput=False)
    qb_d = dp("qb_d", [CB, NQ], F16, isOutput=False)
    rbt_d = dp("rbt_d", [CB, NQ], F16, isOutput=False)
    selA_d = dp("selA_d", [CA * G, G], F16, isOutput=False)
    selB_d = dp("selB_d", [CB * G, G], F16, isOutput=False)
    out_d = dp("out_d", [128, 1], F32, isOutput=True)

    LD = 6
    NCONST = 2

    with ExitStack() as ex:
        E = ex.enter_context
        qa_s = [[E(nc.sbuf_tensor(f"qa{t}{p}", [CA * G, QF], F16))
                 for p in range(2)] for t in range(2)]
        ra_s = [[E(nc.sbuf_tensor(f"ra{t}{p}", [CA * G, RF], F16))
                 for p in range(2)] for t in range(2)]
        qb_s = [E(nc.sbuf_tensor(f"qb{p}", [CB * G, QF], F16))
                for p in range(2)]
        rbt_s = [E(nc.sbuf_tensor(f"rbt{p}", [CB * G, RF], F16))
                 for p in range(2)]
        da_s = E(nc.sbuf_tensor("da", [CA * G, SF], F16))
        db_s = [E(nc.sbuf_tensor(f"db{i}", [CB * G, 2 * SF], F16))
                for i in range(NDP)]
        sqa_s = [E(nc.sbuf_tensor(f"sqa{i}", [CA * G, SF], F16))
                 for i in range(NSQA)]
        sqb_s = [E(nc.sbuf_tensor(f"sqb{i}", [CB * G, 2 * SF], F16))
                 for i in range(NSQB)]
        kt_s = [E(nc.sbuf_tensor(f"kt{i}", [128, 512], F16)) for i in range(2)]
        acc_s = E(nc.sbuf_tensor("acc", [128, cfg.n_acc], F32))
        res_s = E(nc.sbuf_tensor("res", [128, 1], F32))
        selA_s = E(nc.sbuf_tensor("selA", [CA * G, G], F16))
        selB_s = E(nc.sbuf_tensor("selB", [CB * G, G], F16))
        ps_s = [E(nc.psum_tensor(f"ps{i}", [128, 512], F32))
                for i in range(NPSUM)]

        sL = E(nc.semaphore("sL"))
        sLC = E(nc.semaphore("sLC"))
        sL0 = E(nc.semaphore("sL0"))
        sL1 = E(nc.semaphore("sL1"))
        sG = E(nc.semaphore("sG"))
        sV = E(nc.semaphore("sV"))    # DVE subB done (1/producing slot)
        sVq = E(nc.semaphore("sVq"))  # DVE sqA done (1/slot) + final reduce
        sPl = E(nc.semaphore("sPl"))  # Pool subB done (1/producing slot)
        sAq = E(nc.semaphore("sAq"))  # Act sqB pair done (1/pair)
        sP = E(nc.semaphore("sP"))    # PE batch done (1/batch)
        sA = E(nc.semaphore("sA"))    # Act exp units done (1/unit)
        blk = E(nc.Block())

        # access-pattern builders ------------------------------------------
        def q_ap(tile, pcnt, dy, dx):
            off = (R + dy) * WBH + (R + dx)
            return _apv(tile.ap(), 0, pcnt, [[WBH, HS], [1, WB]], off)

        def r_ap(tile, pcnt):
            return _apv(tile.ap(), 0, pcnt, [[WBH, HS], [1, WB]], R)

        def compact2(tile, pcnt, off=0):
            return _apv(tile.ap(), 0, pcnt, [[WB, HS], [1, WB]], off)

        def stream(tile, pcnt, n=SF, off=0):
            return _apv(tile.ap(), 0, pcnt, [[1, n]], off)

        def rgbref_ap(t, ph):
            return (q_ap(qb_s[ph], CB * G, 0, 0) if t == 0
                    else r_ap(rbt_s[ph], CB * G))

        def db_half(gpi, J):
            return compact2(db_s[gpi % NDP], CB * G, (J % 2) * SF)

        @blk.gpsimd
        def _(gp):
            gp.memset(acc_s.ap(), 0.0)
            gp.memset(res_s.ap(), 0.0)
            gp.drain()
            gp.sem_inc(sG, 1)
            seen_pair = -1
            for s in range(NSLAB):
                ph = s % 2
                sLs = sL0 if ph == 0 else sL1
                gp.wait_ge(sLs, 16 * LD * (s // 2 + 1))
                for j5 in range(NS):
                    J = s * NS + j5
                    if cfg.b_dve[J]:
                        continue
                    gpi = J // 2
                    t, dy, dx = cfg.slots[j5]
                    if gpi != seen_pair and gpi - NDP + 1 >= 1:
                        gp.wait_ge(sAq, gpi - NDP + 1)
                    seen_pair = gpi
                    nc.gpsimd.tensor_tensor(
                        db_half(gpi, J),
                        rgbref_ap(t, ph),
                        q_ap(qb_s[ph], CB * G, dy, dx),
                        AluOpType.subtract)
                    gp.drain()
                    gp.sem_inc(sPl, 1)

        @blk.sync
        def _(sp):
            sp.dma_start(selA_s[:], selA_d[:]).then_inc(sLC, 16)
            sp.dma_start(selB_s[:], selB_d[:]).then_inc(sLC, 16)
            for s in range(NSLAB):
                ph = s % 2
                if s >= 2:
                    sp.wait_ge(sVq, NS * (s - 1))
                    pc = cfg.cnt_p[NS * (s - 1) - 1]
                    if pc >= 1:
                        sp.wait_ge(sPl, pc)
                r0 = s * HS
                sLs = sL0 if ph == 0 else sL1
                for t in range(2):
                    sp.dma_start(
                        qa_s[t][ph].ap(),
                        _dram_ap(qa_d, t * CA * NQ + r0 * WBH,
                                 [[NQ, CA], [HpW, G], [1, QF]])
                    ).then_inc(sLs, 16)
                    sp.dma_start(
                        ra_s[t][ph].ap(),
                        _dram_ap(ra_d, t * CA * NQ + (r0 + R) * WBH,
                                 [[NQ, CA], [HpW, G], [1, RF]])
                    ).then_inc(sLs, 16)
                sp.dma_start(
                    qb_s[ph].ap(),
                    _dram_ap(qb_d, r0 * WBH, [[NQ, CB], [HpW, G], [1, QF]])
                ).then_inc(sLs, 16)
                sp.dma_start(
                    rbt_s[ph].ap(),
                    _dram_ap(rbt_d, (r0 + R) * WBH,
                             [[NQ, CB], [HpW, G], [1, RF]])
                ).then_inc(sLs, 16)
            sp.wait_ge(sVq, cfg.TOTS + 1)
            sp.dma_start(out_d[:], res_s.ap()).then_inc(sL, 16)

        @blk.vector
        def _(ve):
            for s in range(NSLAB):
                ph = s % 2
                sLs = sL0 if ph == 0 else sL1
                ve.wait_ge(sLs, 16 * LD * (s // 2 + 1))
                for j5 in range(NS):
                    J = s * NS + j5
                    gpi = J // 2
                    t, dy, dx = cfg.slots[j5]
                    if J - NSQA >= 0:
                        ve.wait_ge(sP, cfg.slot_batch(J - NSQA) + 1)
                    if cfg.b_dve[J]:
                        if gpi - NDP + 1 >= 1:
                            ve.wait_ge(sAq, gpi - NDP + 1)
                        nc.vector.tensor_tensor(
                            db_half(gpi, J),
                            rgbref_ap(t, ph),
                            q_ap(qb_s[ph], CB * G, dy, dx),
                            AluOpType.subtract).then_inc(sV, 1)
                    nc.vector.tensor_tensor(
                        compact2(da_s, CA * G),
                        r_ap(ra_s[t][ph], CA * G),
                        q_ap(qa_s[t][ph], CA * G, dy, dx),
                        AluOpType.subtract)
                    nc.vector.tensor_mul(
                        stream(sqa_s[J % NSQA], CA * G),
                        stream(da_s, CA * G),
                        stream(da_s, CA * G)).then_inc(sVq, 1)
            ve.wait_ge(sA, cfg.TOTB * NC)
            nc.vector.tensor_reduce(
                res_s.ap(), acc_s.ap(), axis=mybir.AxisListType.X,
                op=AluOpType.add).then_inc(sVq, 1)

        @blk.tensor
        def _(pe):
            pe.wait_ge(sLC, 16 * NCONST)
            for s in range(NSLAB):
                for b in range(NB):
                    gb = s * NB + b
                    L = len(cfg.batches[b])
                    gJ0 = s * NS + b * SBATCH
                    if NC * gb - NPSUM + NC >= 1:
                        pe.wait_ge(sA, NC * gb - NPSUM + NC)
                    for jj in range(L):
                        J = gJ0 + jj
                        pe.wait_ge(sVq, J + 1)
                        for c, (co, cn) in enumerate(cfg.chunks):
                            u = gb * NC + c
                            nc.tensor.matmul(
                                ps_s[u % NPSUM][G * jj:G * (jj + 1), :cn],
                                selA_s[:],
                                stream(sqa_s[J % NSQA], CA * G, cn, co),
                                start=True, stop=False, skip_group_check=True,
                                tile_position=(0, G * jj))
                    for jj in range(L):
                        J = gJ0 + jj
                        if jj % 2 == 0:
                            pe.wait_ge(sAq, J // 2 + 1)
                        for c, (co, cn) in enumerate(cfg.chunks):
                            u = gb * NC + c
                            mm = nc.tensor.matmul(
                                ps_s[u % NPSUM][G * jj:G * (jj + 1), :cn],
                                selB_s[:],
                                stream(sqb_s[(J // 2) % NSQB], CB * G, cn,
                                       (J % 2) * SF + co),
                                start=False, stop=True, skip_group_check=True,
                                tile_position=(0, G * jj))
                            if jj == L - 1 and c == NC - 1:
                                mm.then_inc(sP, 1)

        @blk.scalar
        def _(ac):
            ac.wait_ge(sG, 1)
            for s in range(NSLAB):
                for b in range(NB):
                    gb = s * NB + b
                    L = len(cfg.batches[b])
                    gJ0 = s * NS + b * SBATCH
                    for k in range(L // 2):
                        gpi = gJ0 // 2 + k
                        for J in (2 * gpi, 2 * gpi + 1):
                            if cfg.b_dve[J]:
                                ac.wait_ge(sV, cfg.cnt_v[J])
                            else:
                                ac.wait_ge(sPl, cfg.cnt_p[J])
                        if gpi - NSQB >= 0:
                            ac.wait_ge(
                                sP, cfg.slot_batch(2 * (gpi - NSQB) + 1) + 1)
                        nc.scalar.activation(
                            stream(sqb_s[gpi % NSQB], CB * G, 2 * SF),
                            stream(db_s[gpi % NDP], CB * G, 2 * SF),
                            Act.Square).then_inc(sAq, 1)
                    pb = G * L
                    ac.wait_ge(sP, gb + 1)
                    for c, (co, cn) in enumerate(cfg.chunks):
                        u = gb * NC + c
                        nc.scalar.activation(
                            kt_s[u % 2][:pb, :cn], ps_s[u % NPSUM][:pb, :cn],
                            Act.Exp, scale=EXP_SCALE,
                            accum_out=acc_s[:pb, u:u + 1]).then_inc(sA, 1)
    return nc


# ---------------- host side ----------------

def _block_q(plane, cfg):
    """[H, W] -> flat blocked+haloed [G*Hp*WBH] fp16, zero-padded borders."""
    p = np.zeros((cfg.Hp, cfg.W + 2 * R), dtype=np.float32)
    p[R:R + cfg.H, R:R + cfg.W] = plane
    out = np.empty((G, cfg.Hp, cfg.WBH), dtype=np.float16)
    for g in range(G):
        out[g] = p[:, g * cfg.WB:g * cfg.WB + cfg.WBH]
    return np.ascontiguousarray(out).reshape(-1)


def host_precompute(rgb, depth, depth_gt, depth_mask, depth_gt_mask,
                    xy1_grid, Ts, cfg, b):
    tb = b ^ 1
    xy1 = np.asarray(xy1_grid[b], np.float32)
    dep = np.asarray(depth[b, 0], np.float32)
    dgt_b = np.asarray(depth_gt[b, 0], np.float32)
    dgt_t = np.asarray(depth_gt[tb, 0], np.float32)
    mp = np.asarray(depth_mask[b, 0], np.float32)
    mg_b = np.asarray(depth_gt_mask[b, 0], np.float32)
    mg_t = np.asarray(depth_gt_mask[tb, 0], np.float32)

    xyz_p = xy1 * dep
    T21 = (np.linalg.inv(np.asarray(Ts[tb], np.float64)) @
           np.asarray(Ts[b], np.float64)).astype(np.float32)
    Rm, tv = T21[:3, :3], T21[:3, 3]
    txyz = np.einsum('ij,jhw->ihw', Rm, xyz_p).astype(np.float32) \
        + tv[:, None, None].astype(np.float32)
    pos = (txyz[2] > 0).astype(np.float32) * mp

    qa = np.empty((2, CA, cfg.NQ), np.float16)
    ra = np.empty((2, CA, cfg.NQ), np.float16)
    for c in range(3):
        qa[0, c] = _block_q(xyz_p[c], cfg)
        qa[1, c] = _block_q(txyz[c], cfg)
        ra[0, c] = _block_q(xy1[c] * dgt_b, cfg)
        ra[1, c] = _block_q(xy1[c] * dgt_t, cfg)
    # mask channel: (ra3 - qa3)^2 = 0 iff both masks pass, else >= 400
    qa[0, 3] = -MK * (1.0 - _block_q(mp, cfg))
    qa[1, 3] = -MK * (1.0 - _block_q(pos, cfg))
    ra[0, 3] = MK * (1.0 - _block_q(mg_b, cfg))
    ra[1, 3] = MK * (1.0 - _block_q(mg_t, cfg))
    qb = np.stack([_block_q(np.asarray(rgb[b, c], np.float32), cfg)
                   for c in range(3)])
    rbt = np.stack([_block_q(np.asarray(rgb[tb, c], np.float32), cfg)
                    for c in range(3)])
    return {"qa_d": qa, "ra_d": ra, "qb_d": qb, "rbt_d": rbt,
            "selA_d": make_selA(), "selB_d": make_selB()}


def make_in_maps(rgb, depth, depth_gt, depth_mask, depth_gt_mask, xy1_grid, Ts,
                 cfg, n_cores=8):
    return [host_precompute(rgb, depth, depth_gt, depth_mask, depth_gt_mask,
                            xy1_grid, Ts, cfg, b) for b in range(n_cores)]


_CACHED = {}


def _get_nc(cfg_key=(352, 1216, 32)):
    if cfg_key not in _CACHED:
        cfg = Cfg(*cfg_key)
        nc = bass.Bass()
        emit(nc, cfg)
        _CACHED[cfg_key] = (nc, cfg)
    return _CACHED[cfg_key]


def kernel(rgb, depth, depth_gt, depth_mask, depth_gt_mask, xy1_grid, Ts,
           **run_kwargs):
    from concourse.bass_utils import run_bass_kernel_spmd
    nc, cfg = _get_nc()
    maps = make_in_maps(rgb, depth, depth_gt, depth_mask, depth_gt_mask,
                        xy1_grid, Ts, cfg)
    res = run_bass_kernel_spmd(nc, maps, list(range(8)), **run_kwargs)
    total = np.float64(0.0)
    for r in res.results:
        total += np.float64(r["out_d"][:, 0].sum())
    n_gt = max(np.asarray(depth_gt_mask, np.float64).sum(), 1.0)
    loss = -total / n_gt
    kernel.last_results = res
    return np.float32(loss)


# revision 6
# speedup vs baseline: 1.1637x; 1.1637x over previous
"""C3DLoss kernel for Trainium2 — 8-core batch-parallel, raw-Bass implementation.

Per core = one batch frame b (tgt pairing partner tb = b^1):
    partial = sum over both terms (same-frame, cross-frame), all 25 shifts
              delta in [-2,2]^2, all pixels p of
        mref(p) * mq(p+delta) * exp(-50*(|xyz_r(p)-xyz_q(p+d)|^2
                                         + |rgb_r(p)-rgb_q(p+d)|^2))
    loss = -(sum of partials) / max(sum(depth_gt_mask), 1)

v4 design notes (measured-HW-behavior driven):
  - Two-input DVE tensor_tensor runs at 1 elem/cycle/lane regardless of
    layout (the 2x_1p fast mode only engages for single-stream ops), so
    subtracts use COMPACT 2-D access patterns (1216 useful cols, no halo
    waste).  Squares read one stream (mult(d,d)) as a contiguous 1-D AP
    and do hit 2x.  GpSimd wants 2-D patterns (its 8 Q7 cores
    parallelize across rows; a long 1-D run serializes 2.3x slower).
  - fp16 planes throughout; masks folded into feature channel 3
    (ra3=+20*(1-mref), qa3=-20*(1-mq)) so one selA matmul pass covers
    xyz+mask and one selB pass covers rgb.
  - Per-slot split: DVE does subA always, sqA always, subB on every 8th
    slot; GpSimd does the other 7/8 of subB; ScalarE squares rgb diffs
    in slot-pairs and runs exp+accumulate from PSUM.
  - PE per 4-slot batch: all selA matmuls, then all selB (weight reuse),
    compact contiguous rhs chunks, 8 PSUM banks, then_inc signaling.
"""

import sys

for _p in ("/opt/trn_rl_repo", "/opt/pypackages"):
    if _p not in sys.path:
        sys.path.insert(0, _p)

from contextlib import ExitStack

import numpy as np

import concourse.bass as bass
import concourse.mybir as mybir
from concourse.ap import AP
from concourse.alu_op_type import AluOpType

F32 = mybir.dt.float32
F16 = mybir.dt.float16

R = 2
G = 32            # W-blocks; one shift-slot = 32 PSUM partitions
CA = 4            # tile A channels: x, y, z, mask
CB = 3            # tile B channels: r, g, b
SBATCH = 4        # shift slots per 128-partition PSUM bank
NPSUM = 8         # rotating PSUM banks (unit = (batch, chunk))
NSQA = 8          # rotating sqa buffers (per slot)
NDP = 4           # rotating db pair buffers (per 2 slots)
NSQB = 4          # rotating sqb pair buffers (per 2 slots)
DVE_B_EVERY = 2   # DVE handles subB on slots J % DVE_B_EVERY == 0
MK = 20.0         # mask channel scale; (2*MK)^2 = 1600 >> 1/50
EXP_SCALE = -50.0


class Cfg:
    def __init__(self, H=352, W=1216, HS=32):
        assert W % G == 0 and H % HS == 0
        self.H, self.W, self.HS = H, W, HS
        self.WB = W // G                      # 38
        self.WBH = self.WB + 2 * R            # 42
        self.Hp = H + 2 * R                   # 356
        self.NSLAB = H // HS                  # 11
        self.NQ = G * self.Hp * self.WBH      # haloed plane elems
        self.QF = (HS + 2 * R) * self.WBH     # query tile free size 1512
        self.RF = HS * self.WBH               # ref tile free size 1344
        self.SF = HS * self.WB                # compact slot size 1216
        # col-chunks of the compact slot (PSUM bank <= 512 f32/partition)
        cw = (512 // self.WB) * self.WB       # 494
        self.chunks = []
        o = 0
        while o < self.SF:
            self.chunks.append((o, min(cw, self.SF - o)))
            o += cw
        self.NC = len(self.chunks)            # 3
        self.slots = [(t, dy, dx) for t in (0, 1)
                      for dy in range(-R, R + 1) for dx in range(-R, R + 1)]
        self.NS = len(self.slots)             # 50
        assert self.NS % 2 == 0
        self.batches = [self.slots[i:i + SBATCH]
                        for i in range(0, self.NS, SBATCH)]
        self.NB = len(self.batches)           # 13
        self.TOTS = self.NSLAB * self.NS      # 550 slots
        self.TOTB = self.NSLAB * self.NB      # 143 batches
        self.n_acc = self.TOTB * self.NC      # 429 acc columns
        # per-slot subB producer bookkeeping
        self.b_dve = [J % DVE_B_EVERY == 0 for J in range(self.TOTS)]
        # cumulative producer instruction counts AFTER slot J's subB
        self.cnt_v = []   # DVE subB count
        self.cnt_p = []   # Pool subB count
        cv = cp = 0
        for J in range(self.TOTS):
            if self.b_dve[J]:
                cv += 1
            else:
                cp += 1
            self.cnt_v.append(cv)
            self.cnt_p.append(cp)

    def slot_batch(self, J):
        return (J // self.NS) * self.NB + (J % self.NS) // SBATCH


def _apv(t_ap, p0, pcnt, free_dims, free_off=0):
    pstride = t_ap.ap[0][0]
    base = t_ap.offset + p0 * pstride + free_off
    return AP(t_ap.tensor, base, [[pstride, pcnt]] + [list(d) for d in free_dims])


def _dram_ap(handle, offset, dims):
    a = handle[:]
    return AP(a.tensor, a.offset + offset, [list(d) for d in dims])


def make_selA():
    s = np.zeros((CA * G, G), dtype=np.float16)
    for c in range(CA):
        for g in range(G):
            s[c * G + g, g] = 1
    return s


def make_selB():
    s = np.zeros((CB * G, G), dtype=np.float16)
    for c in range(CB):
        for g in range(G):
            s[c * G + g, g] = 1
    return s


def emit(nc: bass.Bass, cfg: Cfg):
    HS, WB, WBH, Hp = cfg.HS, cfg.WB, cfg.WBH, cfg.Hp
    NQ, QF, RF, SF = cfg.NQ, cfg.QF, cfg.RF, cfg.SF
    NSLAB, NB, NC, NS = cfg.NSLAB, cfg.NB, cfg.NC, cfg.NS
    Act = mybir.ActivationFunctionType
    HpW = Hp * WBH

    dp = nc.declare_dram_parameter
    qa_d = dp("qa_d", [2, CA, NQ], F16, isOutput=False)
    ra_d = dp("ra_d", [2, CA, NQ], F16, isOutput=False)
    qb_d = dp("qb_d", [CB, NQ], F16, isOutput=False)
    rbt_d = dp("rbt_d", [CB, NQ], F16, isOutput=False)
    selA_d = dp("selA_d", [CA * G, G], F16, isOutput=False)
    selB_d = dp("selB_d", [CB * G, G], F16, isOutput=False)
    out_d = dp("out_d", [128, 1], F32, isOutput=True)

    LD = 6
    NCONST = 2

    with ExitStack() as ex:
        E = ex.enter_context
        qa_s = [[E(nc.sbuf_tensor(f"qa{t}{p}", [CA * G, QF], F16))
                 for p in range(2)] for t in range(2)]
        ra_s = [[E(nc.sbuf_tensor(f"ra{t}{p}", [CA * G, RF], F16))
                 for p in range(2)] for t in range(2)]
        qb_s = [E(nc.sbuf_tensor(f"qb{p}", [CB * G, QF], F16))
                for p in range(2)]
        rbt_s = [E(nc.sbuf_tensor(f"rbt{p}", [CB * G, RF], F16))
                 for p in range(2)]
        da_s = E(nc.sbuf_tensor("da", [CA * G, SF], F16))
        db_s = [E(nc.sbuf_tensor(f"db{i}", [CB * G, 2 * SF], F16))
                for i in range(NDP)]
        sqa_s = [E(nc.sbuf_tensor(f"sqa{i}", [CA * G, SF], F16))
                 for i in range(NSQA)]
        sqb_s = [E(nc.sbuf_tensor(f"sqb{i}", [CB * G, 2 * SF], F16))
                 for i in range(NSQB)]
        kt_s = [E(nc.sbuf_tensor(f"kt{i}", [128, 512], F16)) for i in range(2)]
        acc_s = E(nc.sbuf_tensor("acc", [128, cfg.n_acc], F32))
        res_s = E(nc.sbuf_tensor("res", [128, 1], F32))
        selA_s = E(nc.sbuf_tensor("selA", [CA * G, G], F16))
        selB_s = E(nc.sbuf_tensor("selB", [CB * G, G], F16))
        ps_s = [E(nc.psum_tensor(f"ps{i}", [128, 512], F32))
                for i in range(NPSUM)]

        sL = E(nc.semaphore("sL"))
        sLC = E(nc.semaphore("sLC"))
        sL0 = E(nc.semaphore("sL0"))
        sL1 = E(nc.semaphore("sL1"))
        sG = E(nc.semaphore("sG"))
        sV = E(nc.semaphore("sV"))    # DVE subB done (1/producing slot)
        sVq = E(nc.semaphore("sVq"))  # DVE sqA done (1/slot) + final reduce
        sPl = E(nc.semaphore("sPl"))  # Pool subB done (1/producing slot)
        sAq = E(nc.semaphore("sAq"))  # Act sqB pair done (1/pair)
        sP = E(nc.semaphore("sP"))    # PE batch done (1/batch)
        sA = E(nc.semaphore("sA"))    # Act exp units done (1/unit)
        blk = E(nc.Block())

        # access-pattern builders ------------------------------------------
        def q_ap(tile, pcnt, dy, dx):
            off = (R + dy) * WBH + (R + dx)
            return _apv(tile.ap(), 0, pcnt, [[WBH, HS], [1, WB]], off)

        def r_ap(tile, pcnt):
            return _apv(tile.ap(), 0, pcnt, [[WBH, HS], [1, WB]], R)

        def compact2(tile, pcnt, off=0):
            return _apv(tile.ap(), 0, pcnt, [[WB, HS], [1, WB]], off)

        def stream(tile, pcnt, n=SF, off=0):
            return _apv(tile.ap(), 0, pcnt, [[1, n]], off)

        def rgbref_ap(t, ph):
            return (q_ap(qb_s[ph], CB * G, 0, 0) if t == 0
                    else r_ap(rbt_s[ph], CB * G))

        def db_half(gpi, J):
            return compact2(db_s[gpi % NDP], CB * G, (J % 2) * SF)

        @blk.gpsimd
        def _(gp):
            gp.memset(acc_s.ap(), 0.0)
            gp.memset(res_s.ap(), 0.0)
            gp.drain()
            gp.sem_inc(sG, 1)
            seen_pair = -1
            for s in range(NSLAB):
                ph = s % 2
                sLs = sL0 if ph == 0 else sL1
                gp.wait_ge(sLs, 16 * LD * (s // 2 + 1))
                for j5 in range(NS):
                    J = s * NS + j5
                    if cfg.b_dve[J]:
                        continue
                    gpi = J // 2
                    t, dy, dx = cfg.slots[j5]
                    if gpi != seen_pair and gpi - NDP + 1 >= 1:
                        gp.wait_ge(sAq, gpi - NDP + 1)
                    seen_pair = gpi
                    nc.gpsimd.tensor_tensor(
                        db_half(gpi, J),
                        rgbref_ap(t, ph),
                        q_ap(qb_s[ph], CB * G, dy, dx),
                        AluOpType.subtract)
                    gp.drain()
                    gp.sem_inc(sPl, 1)

        @blk.sync
        def _(sp):
            sp.dma_start(selA_s[:], selA_d[:]).then_inc(sLC, 16)
            sp.dma_start(selB_s[:], selB_d[:]).then_inc(sLC, 16)
            for s in range(NSLAB):
                ph = s % 2
                if s >= 2:
                    sp.wait_ge(sVq, NS * (s - 1))
                    pc = cfg.cnt_p[NS * (s - 1) - 1]
                    if pc >= 1:
                        sp.wait_ge(sPl, pc)
                r0 = s * HS
                sLs = sL0 if ph == 0 else sL1
                for t in range(2):
                    sp.dma_start(
                        qa_s[t][ph].ap(),
                        _dram_ap(qa_d, t * CA * NQ + r0 * WBH,
                                 [[NQ, CA], [HpW, G], [1, QF]])
                    ).then_inc(sLs, 16)
                    sp.dma_start(
                        ra_s[t][ph].ap(),
                        _dram_ap(ra_d, t * CA * NQ + (r0 + R) * WBH,
                                 [[NQ, CA], [HpW, G], [1, RF]])
                    ).then_inc(sLs, 16)
                sp.dma_start(
                    qb_s[ph].ap(),
                    _dram_ap(qb_d, r0 * WBH, [[NQ, CB], [HpW, G], [1, QF]])
                ).then_inc(sLs, 16)
                sp.dma_start(
                    rbt_s[ph].ap(),
                    _dram_ap(rbt_d, (r0 + R) * WBH,
                             [[NQ, CB], [HpW, G], [1, RF]])
                ).then_inc(sLs, 16)
            sp.wait_ge(sVq, cfg.TOTS + 1)
            sp.dma_start(out_d[:], res_s.ap()).then_inc(sL, 16)

        @blk.vector
        def _(ve):
            for s in range(NSLAB):
                ph = s % 2
                sLs = sL0 if ph == 0 else sL1
                ve.wait_ge(sLs, 16 * LD * (s // 2 + 1))
                for j5 in range(NS):
                    J = s * NS + j5
                    gpi = J // 2
                    t, dy, dx = cfg.slots[j5]
                    if J - NSQA >= 0:
                        ve.wait_ge(sP, cfg.slot_batch(J - NSQA) + 1)
                    if cfg.b_dve[J]:
                        if gpi - NDP + 1 >= 1:
                            ve.wait_ge(sAq, gpi - NDP + 1)
                        nc.vector.tensor_tensor(
                            db_half(gpi, J),
                            rgbref_ap(t, ph),
                            q_ap(qb_s[ph], CB * G, dy, dx),
                            AluOpType.subtract).then_inc(sV, 1)
                    nc.vector.tensor_tensor(
                        compact2(da_s, CA * G),
                        r_ap(ra_s[t][ph], CA * G),
                        q_ap(qa_s[t][ph], CA * G, dy, dx),
                        AluOpType.subtract)
                    nc.vector.tensor_mul(
                        stream(sqa_s[J % NSQA], CA * G),
                        stream(da_s, CA * G),
                        stream(da_s, CA * G)).then_inc(sVq, 1)
            ve.wait_ge(sA, cfg.TOTB * NC)
            nc.vector.tensor_reduce(
                res_s.ap(), acc_s.ap(), axis=mybir.AxisListType.X,
                op=AluOpType.add).then_inc(sVq, 1)

        @blk.tensor
        def _(pe):
            pe.wait_ge(sLC, 16 * NCONST)
            for s in range(NSLAB):
                for b in range(NB):
                    gb = s * NB + b
                    L = len(cfg.batches[b])
                    gJ0 = s * NS + b * SBATCH
                    if NC * gb - NPSUM + NC >= 1:
                        pe.wait_ge(sA, NC * gb - NPSUM + NC)
                    for jj in range(L):
                        J = gJ0 + jj
                        pe.wait_ge(sVq, J + 1)
                        for c, (co, cn) in enumerate(cfg.chunks):
                            u = gb * NC + c
                            nc.tensor.matmul(
                                ps_s[u % NPSUM][G * jj:G * (jj + 1), :cn],
                                selA_s[:],
                                stream(sqa_s[J % NSQA], CA * G, cn, co),
                                start=True, stop=False, skip_group_check=True,
                                tile_position=(0, G * jj))
                    for jj in range(L):
                        J = gJ0 + jj
                        if jj % 2 == 0:
                            pe.wait_ge(sAq, J // 2 + 1)
                        for c, (co, cn) in enumerate(cfg.chunks):
                            u = gb * NC + c
                            mm = nc.tensor.matmul(
                                ps_s[u % NPSUM][G * jj:G * (jj + 1), :cn],
                                selB_s[:],
                                stream(sqb_s[(J // 2) % NSQB], CB * G, cn,
                                       (J % 2) * SF + co),
                                start=False, stop=True, skip_group_check=True,
                                tile_position=(0, G * jj))
                            if jj == L - 1 and c == NC - 1:
                                mm.then_inc(sP, 1)

        @blk.scalar
        def _(ac):
            ac.wait_ge(sG, 1)
            for s in range(NSLAB):
                for b in range(NB):
                    gb = s * NB + b
                    L = len(cfg.batches[b])
                    gJ0 = s * NS + b * SBATCH
                    for k in range(L // 2):
                        gpi = gJ0 // 2 + k
                        for J in (2 * gpi, 2 * gpi + 1):
                            if cfg.b_dve[J]:
                                ac.wait_ge(sV, cfg.cnt_v[J])
                            else:
                                ac.wait_ge(sPl, cfg.cnt_p[J])
                        if gpi - NSQB >= 0:
                            ac.wait_ge(
                                sP, cfg.slot_batch(2 * (gpi - NSQB) + 1) + 1)
                        nc.scalar.activation(
                            stream(sqb_s[gpi % NSQB], CB * G, 2 * SF),
                            stream(db_s[gpi % NDP], CB * G, 2 * SF),
                            Act.Square).then_inc(sAq, 1)
                    pb = G * L
                    ac.wait_ge(sP, gb + 1)
                    for c, (co, cn) in enumerate(cfg.chunks):
                        u = gb * NC + c
                        nc.scalar.activation(
                            kt_s[u % 2][:pb, :cn], ps_s[u % NPSUM][:pb, :cn],
                            Act.Exp, scale=EXP_SCALE,
                            accum_out=acc_s[:pb, u:u + 1]).then_inc(sA, 1)
    return nc


# ---------------- host side ----------------

def _block_q(plane, cfg):
    """[H, W] -> flat blocked+haloed [G*Hp*WBH] fp16, zero-padded borders."""
    p = np.zeros((cfg.Hp, cfg.W + 2 * R), dtype=np.float32)
    p[R:R + cfg.H, R:R + cfg.W] = plane
    out = np.empty((G, cfg.Hp, cfg.WBH), dtype=np.float16)
    for g in range(G):
        out[g] = p[:, g * cfg.WB:g * cfg.WB + cfg.WBH]
    return np.ascontiguousarray(out).reshape(-1)


def host_precompute(rgb, depth, depth_gt, depth_mask, depth_gt_mask,
                    xy1_grid, Ts, cfg, b):
    tb = b ^ 1
    xy1 = np.asarray(xy1_grid[b], np.float32)
    dep = np.asarray(depth[b, 0], np.float32)
    dgt_b = np.asarray(depth_gt[b, 0], np.float32)
    dgt_t = np.asarray(depth_gt[tb, 0], np.float32)
    mp = np.asarray(depth_mask[b, 0], np.float32)
    mg_b = np.asarray(depth_gt_mask[b, 0], np.float32)
    mg_t = np.asarray(depth_gt_mask[tb, 0], np.float32)

    xyz_p = xy1 * dep
    T21 = (np.linalg.inv(np.asarray(Ts[tb], np.float64)) @
           np.asarray(Ts[b], np.float64)).astype(np.float32)
    Rm, tv = T21[:3, :3], T21[:3, 3]
    txyz = np.einsum('ij,jhw->ihw', Rm, xyz_p).astype(np.float32) \
        + tv[:, None, None].astype(np.float32)
    pos = (txyz[2] > 0).astype(np.float32) * mp

    qa = np.empty((2, CA, cfg.NQ), np.float16)
    ra = np.empty((2, CA, cfg.NQ), np.float16)
    for c in range(3):
        qa[0, c] = _block_q(xyz_p[c], cfg)
        qa[1, c] = _block_q(txyz[c], cfg)
        ra[0, c] = _block_q(xy1[c] * dgt_b, cfg)
        ra[1, c] = _block_q(xy1[c] * dgt_t, cfg)
    # mask channel: (ra3 - qa3)^2 = 0 iff both masks pass, else >= 400
    qa[0, 3] = -MK * (1.0 - _block_q(mp, cfg))
    qa[1, 3] = -MK * (1.0 - _block_q(pos, cfg))
    ra[0, 3] = MK * (1.0 - _block_q(mg_b, cfg))
    ra[1, 3] = MK * (1.0 - _block_q(mg_t, cfg))
    qb = np.stack([_block_q(np.asarray(rgb[b, c], np.float32), cfg)
                   for c in range(3)])
    rbt = np.stack([_block_q(np.asarray(rgb[tb, c], np.float32), cfg)
                    for c in range(3)])
    return {"qa_d": qa, "ra_d": ra, "qb_d": qb, "rbt_d": rbt,
            "selA_d": make_selA(), "selB_d": make_selB()}


def make_in_maps(rgb, depth, depth_gt, depth_mask, depth_gt_mask, xy1_grid, Ts,
                 cfg, n_cores=8):
    return [host_precompute(rgb, depth, depth_gt, depth_mask, depth_gt_mask,
                            xy1_grid, Ts, cfg, b) for b in range(n_cores)]


_CACHED = {}


def _get_nc(cfg_key=(352, 1216, 32)):
    if cfg_key not in _CACHED:
        cfg = Cfg(*cfg_key)
        nc = bass.Bass()
        emit(nc, cfg)
        _CACHED[cfg_key] = (nc, cfg)
    return _CACHED[cfg_key]


def kernel(rgb, depth, depth_gt, depth_mask, depth_gt_mask, xy1_grid, Ts,
           **run_kwargs):
    from concourse.bass_utils import run_bass_kernel_spmd
    nc, cfg = _get_nc()
    maps = make_in_maps(rgb, depth, depth_gt, depth_mask, depth_gt_mask,
                        xy1_grid, Ts, cfg)
    res = run_bass_kernel_spmd(nc, maps, list(range(8)), **run_kwargs)
    total = np.float64(0.0)
    for r in res.results:
        total += np.float64(r["out_d"][:, 0].sum())
    n_gt = max(np.asarray(depth_gt_mask, np.float64).sum(), 1.0)
    loss = -total / n_gt
    kernel.last_results = res
    return np.float32(loss)


# revision 7
# speedup vs baseline: 1.5727x; 1.3515x over previous
"""C3DLoss kernel for Trainium2 — 8-core batch-parallel, raw-Bass implementation.

Per core = one batch frame b (tgt pairing partner tb = b^1):
    partial = sum over both terms (same-frame, cross-frame), all 25 shifts
              delta in [-2,2]^2, all pixels p of
        mref(p) * mq(p+delta) * exp(-50*(|xyz_r(p)-xyz_q(p+d)|^2
                                         + |rgb_r(p)-rgb_q(p+d)|^2))
    loss = -(sum of partials) / max(sum(depth_gt_mask), 1)

v4 design notes (measured-HW-behavior driven):
  - Two-input DVE tensor_tensor runs at 1 elem/cycle/lane regardless of
    layout (the 2x_1p fast mode only engages for single-stream ops), so
    subtracts use COMPACT 2-D access patterns (1216 useful cols, no halo
    waste).  Squares read one stream (mult(d,d)) as a contiguous 1-D AP
    and do hit 2x.  GpSimd wants 2-D patterns (its 8 Q7 cores
    parallelize across rows; a long 1-D run serializes 2.3x slower).
  - fp16 planes throughout; masks folded into feature channel 3
    (ra3=+20*(1-mref), qa3=-20*(1-mq)) so one selA matmul pass covers
    xyz+mask and one selB pass covers rgb.
  - Per-slot split: DVE does subA always, sqA always, subB on every 8th
    slot; GpSimd does the other 7/8 of subB; ScalarE squares rgb diffs
    in slot-pairs and runs exp+accumulate from PSUM.
  - PE per 4-slot batch: all selA matmuls, then all selB (weight reuse),
    compact contiguous rhs chunks, 8 PSUM banks, then_inc signaling.
"""

import sys

for _p in ("/opt/trn_rl_repo", "/opt/pypackages"):
    if _p not in sys.path:
        sys.path.insert(0, _p)

from contextlib import ExitStack

import numpy as np

import concourse.bass as bass
import concourse.mybir as mybir
from concourse.ap import AP
from concourse.alu_op_type import AluOpType

F32 = mybir.dt.float32
F16 = mybir.dt.float16

R = 2
G = 32            # W-blocks; one shift-slot = 32 PSUM partitions
CA = 4            # tile A channels: x, y, z, mask
CB = 3            # tile B channels: r, g, b
SBATCH = 4        # shift slots per 128-partition PSUM bank
NPSUM = 8         # rotating PSUM banks (unit = (batch, chunk))
NSQA = 8          # rotating sqa buffers (per slot)
NDP = 4           # rotating db pair buffers (per 2 slots)
NSQB = 4          # rotating sqb pair buffers (per 2 slots)
DVE_B_EVERY = 1   # DVE handles subB on slots J % DVE_B_EVERY == 0
MK = 20.0         # mask channel scale; (2*MK)^2 = 1600 >> 1/50
EXP_SCALE = -50.0


class Cfg:
    def __init__(self, H=352, W=1216, HS=32):
        assert W % G == 0 and H % HS == 0
        self.H, self.W, self.HS = H, W, HS
        self.WB = W // G                      # 38
        self.WBH = self.WB + 2 * R            # 42
        self.Hp = H + 2 * R                   # 356
        self.NSLAB = H // HS                  # 11
        self.NQ = G * self.Hp * self.WBH      # haloed plane elems
        self.QF = (HS + 2 * R) * self.WBH     # query tile free size 1512
        self.RF = HS * self.WBH               # ref tile free size 1344
        self.SF = HS * self.WB                # compact slot size 1216
        # col-chunks of the compact slot (PSUM bank <= 512 f32/partition)
        cw = (512 // self.WB) * self.WB       # 494
        self.chunks = []
        o = 0
        while o < self.SF:
            self.chunks.append((o, min(cw, self.SF - o)))
            o += cw
        self.NC = len(self.chunks)            # 3
        self.slots = [(t, dy, dx) for t in (0, 1)
                      for dy in range(-R, R + 1) for dx in range(-R, R + 1)]
        self.NS = len(self.slots)             # 50
        assert self.NS % 2 == 0
        self.batches = [self.slots[i:i + SBATCH]
                        for i in range(0, self.NS, SBATCH)]
        self.NB = len(self.batches)           # 13
        self.TOTS = self.NSLAB * self.NS      # 550 slots
        self.TOTB = self.NSLAB * self.NB      # 143 batches
        self.n_acc = self.TOTB * self.NC      # 429 acc columns
        # per-slot subB producer bookkeeping
        self.b_dve = [J % DVE_B_EVERY == 0 for J in range(self.TOTS)]
        # cumulative producer instruction counts AFTER slot J's subB
        self.cnt_v = []   # DVE subB count
        self.cnt_p = []   # Pool subB count
        cv = cp = 0
        for J in range(self.TOTS):
            if self.b_dve[J]:
                cv += 1
            else:
                cp += 1
            self.cnt_v.append(cv)
            self.cnt_p.append(cp)

    def slot_batch(self, J):
        return (J // self.NS) * self.NB + (J % self.NS) // SBATCH


def _apv(t_ap, p0, pcnt, free_dims, free_off=0):
    pstride = t_ap.ap[0][0]
    base = t_ap.offset + p0 * pstride + free_off
    return AP(t_ap.tensor, base, [[pstride, pcnt]] + [list(d) for d in free_dims])


def _dram_ap(handle, offset, dims):
    a = handle[:]
    return AP(a.tensor, a.offset + offset, [list(d) for d in dims])


def make_selA():
    s = np.zeros((CA * G, G), dtype=np.float16)
    for c in range(CA):
        for g in range(G):
            s[c * G + g, g] = 1
    return s


def make_selB():
    s = np.zeros((CB * G, G), dtype=np.float16)
    for c in range(CB):
        for g in range(G):
            s[c * G + g, g] = 1
    return s


def emit(nc: bass.Bass, cfg: Cfg):
    HS, WB, WBH, Hp = cfg.HS, cfg.WB, cfg.WBH, cfg.Hp
    NQ, QF, RF, SF = cfg.NQ, cfg.QF, cfg.RF, cfg.SF
    NSLAB, NB, NC, NS = cfg.NSLAB, cfg.NB, cfg.NC, cfg.NS
    Act = mybir.ActivationFunctionType
    HpW = Hp * WBH

    dp = nc.declare_dram_parameter
    qa_d = dp("qa_d", [2, CA, NQ], F16, isOutput=False)
    ra_d = dp("ra_d", [2, CA, NQ], F16, isOutput=False)
    qb_d = dp("qb_d", [CB, NQ], F16, isOutput=False)
    rbt_d = dp("rbt_d", [CB, NQ], F16, isOutput=False)
    selA_d = dp("selA_d", [CA * G, G], F16, isOutput=False)
    selB_d = dp("selB_d", [CB * G, G], F16, isOutput=False)
    out_d = dp("out_d", [128, 1], F32, isOutput=True)

    LD = 6
    NCONST = 2

    with ExitStack() as ex:
        E = ex.enter_context
        qa_s = [[E(nc.sbuf_tensor(f"qa{t}{p}", [CA * G, QF], F16))
                 for p in range(2)] for t in range(2)]
        ra_s = [[E(nc.sbuf_tensor(f"ra{t}{p}", [CA * G, RF], F16))
                 for p in range(2)] for t in range(2)]
        qb_s = [E(nc.sbuf_tensor(f"qb{p}", [CB * G, QF], F16))
                for p in range(2)]
        rbt_s = [E(nc.sbuf_tensor(f"rbt{p}", [CB * G, RF], F16))
                 for p in range(2)]
        da_s = E(nc.sbuf_tensor("da", [CA * G, SF], F16))
        db_s = [E(nc.sbuf_tensor(f"db{i}", [CB * G, 2 * SF], F16))
                for i in range(NDP)]
        sqa_s = [E(nc.sbuf_tensor(f"sqa{i}", [CA * G, SF], F16))
                 for i in range(NSQA)]
        sqb_s = [E(nc.sbuf_tensor(f"sqb{i}", [CB * G, 2 * SF], F16))
                 for i in range(NSQB)]
        kt_s = [E(nc.sbuf_tensor(f"kt{i}", [128, 512], F16)) for i in range(2)]
        acc_s = E(nc.sbuf_tensor("acc", [128, cfg.n_acc], F32))
        res_s = E(nc.sbuf_tensor("res", [128, 1], F32))
        selA_s = E(nc.sbuf_tensor("selA", [CA * G, G], F16))
        selB_s = E(nc.sbuf_tensor("selB", [CB * G, G], F16))
        ps_s = [E(nc.psum_tensor(f"ps{i}", [128, 512], F32))
                for i in range(NPSUM)]

        sL = E(nc.semaphore("sL"))
        sLC = E(nc.semaphore("sLC"))
        sL0 = E(nc.semaphore("sL0"))
        sL1 = E(nc.semaphore("sL1"))
        sG = E(nc.semaphore("sG"))
        sV = E(nc.semaphore("sV"))    # DVE subB done (1/producing slot)
        sVq = E(nc.semaphore("sVq"))  # DVE sqA done (1/slot) + final reduce
        sPl = E(nc.semaphore("sPl"))  # Pool subB done (1/producing slot)
        sAq = E(nc.semaphore("sAq"))  # Act sqB pair done (1/pair)
        sP = E(nc.semaphore("sP"))    # PE batch done (1/batch)
        sA = E(nc.semaphore("sA"))    # Act exp units done (1/unit)
        blk = E(nc.Block())

        # access-pattern builders ------------------------------------------
        def q_ap(tile, pcnt, dy, dx):
            off = (R + dy) * WBH + (R + dx)
            return _apv(tile.ap(), 0, pcnt, [[WBH, HS], [1, WB]], off)

        def r_ap(tile, pcnt):
            return _apv(tile.ap(), 0, pcnt, [[WBH, HS], [1, WB]], R)

        def compact2(tile, pcnt, off=0):
            return _apv(tile.ap(), 0, pcnt, [[WB, HS], [1, WB]], off)

        def stream(tile, pcnt, n=SF, off=0):
            return _apv(tile.ap(), 0, pcnt, [[1, n]], off)

        def rgbref_ap(t, ph):
            return (q_ap(qb_s[ph], CB * G, 0, 0) if t == 0
                    else r_ap(rbt_s[ph], CB * G))

        def db_half(gpi, J):
            return compact2(db_s[gpi % NDP], CB * G, (J % 2) * SF)

        @blk.gpsimd
        def _(gp):
            gp.memset(acc_s.ap(), 0.0)
            gp.memset(res_s.ap(), 0.0)
            gp.drain()
            gp.sem_inc(sG, 1)
            seen_pair = -1
            for s in range(NSLAB):
                ph = s % 2
                sLs = sL0 if ph == 0 else sL1
                gp.wait_ge(sLs, 16 * LD * (s // 2 + 1))
                for j5 in range(NS):
                    J = s * NS + j5
                    if cfg.b_dve[J]:
                        continue
                    gpi = J // 2
                    t, dy, dx = cfg.slots[j5]
                    if gpi != seen_pair and gpi - NDP + 1 >= 1:
                        gp.wait_ge(sAq, gpi - NDP + 1)
                    seen_pair = gpi
                    nc.gpsimd.tensor_tensor(
                        db_half(gpi, J),
                        rgbref_ap(t, ph),
                        q_ap(qb_s[ph], CB * G, dy, dx),
                        AluOpType.subtract)
                    gp.drain()
                    gp.sem_inc(sPl, 1)

        @blk.sync
        def _(sp):
            sp.dma_start(selA_s[:], selA_d[:]).then_inc(sLC, 16)
            sp.dma_start(selB_s[:], selB_d[:]).then_inc(sLC, 16)
            for s in range(NSLAB):
                ph = s % 2
                if s >= 2:
                    sp.wait_ge(sVq, NS * (s - 1))
                    pc = cfg.cnt_p[NS * (s - 1) - 1]
                    if pc >= 1:
                        sp.wait_ge(sPl, pc)
                r0 = s * HS
                sLs = sL0 if ph == 0 else sL1
                for t in range(2):
                    sp.dma_start(
                        qa_s[t][ph].ap(),
                        _dram_ap(qa_d, t * CA * NQ + r0 * WBH,
                                 [[NQ, CA], [HpW, G], [1, QF]])
                    ).then_inc(sLs, 16)
                    sp.dma_start(
                        ra_s[t][ph].ap(),
                        _dram_ap(ra_d, t * CA * NQ + (r0 + R) * WBH,
                                 [[NQ, CA], [HpW, G], [1, RF]])
                    ).then_inc(sLs, 16)
                sp.dma_start(
                    qb_s[ph].ap(),
                    _dram_ap(qb_d, r0 * WBH, [[NQ, CB], [HpW, G], [1, QF]])
                ).then_inc(sLs, 16)
                sp.dma_start(
                    rbt_s[ph].ap(),
                    _dram_ap(rbt_d, (r0 + R) * WBH,
                             [[NQ, CB], [HpW, G], [1, RF]])
                ).then_inc(sLs, 16)
            sp.wait_ge(sVq, cfg.TOTS + 1)
            sp.dma_start(out_d[:], res_s.ap()).then_inc(sL, 16)

        @blk.vector
        def _(ve):
            for s in range(NSLAB):
                ph = s % 2
                sLs = sL0 if ph == 0 else sL1
                ve.wait_ge(sLs, 16 * LD * (s // 2 + 1))
                for j5 in range(NS):
                    J = s * NS + j5
                    gpi = J // 2
                    t, dy, dx = cfg.slots[j5]
                    if J - NSQA >= 0:
                        ve.wait_ge(sP, cfg.slot_batch(J - NSQA) + 1)
                    if cfg.b_dve[J]:
                        if gpi - NDP + 1 >= 1:
                            ve.wait_ge(sAq, gpi - NDP + 1)
                        nc.vector.tensor_tensor(
                            db_half(gpi, J),
                            rgbref_ap(t, ph),
                            q_ap(qb_s[ph], CB * G, dy, dx),
                            AluOpType.subtract).then_inc(sV, 1)
                    nc.vector.tensor_tensor(
                        compact2(da_s, CA * G),
                        r_ap(ra_s[t][ph], CA * G),
                        q_ap(qa_s[t][ph], CA * G, dy, dx),
                        AluOpType.subtract)
                    nc.vector.tensor_mul(
                        stream(sqa_s[J % NSQA], CA * G),
                        stream(da_s, CA * G),
                        stream(da_s, CA * G)).then_inc(sVq, 1)
            ve.wait_ge(sA, cfg.TOTB * NC)
            nc.vector.tensor_reduce(
                res_s.ap(), acc_s.ap(), axis=mybir.AxisListType.X,
                op=AluOpType.add).then_inc(sVq, 1)

        @blk.tensor
        def _(pe):
            pe.wait_ge(sLC, 16 * NCONST)
            for s in range(NSLAB):
                for b in range(NB):
                    gb = s * NB + b
                    L = len(cfg.batches[b])
                    gJ0 = s * NS + b * SBATCH
                    if NC * gb - NPSUM + NC >= 1:
                        pe.wait_ge(sA, NC * gb - NPSUM + NC)
                    for jj in range(L):
                        J = gJ0 + jj
                        pe.wait_ge(sVq, J + 1)
                        for c, (co, cn) in enumerate(cfg.chunks):
                            u = gb * NC + c
                            nc.tensor.matmul(
                                ps_s[u % NPSUM][G * jj:G * (jj + 1), :cn],
                                selA_s[:],
                                stream(sqa_s[J % NSQA], CA * G, cn, co),
                                start=True, stop=False, skip_group_check=True,
                                tile_position=(0, G * jj))
                    for jj in range(L):
                        J = gJ0 + jj
                        if jj % 2 == 0:
                            pe.wait_ge(sAq, J // 2 + 1)
                        for c, (co, cn) in enumerate(cfg.chunks):
                            u = gb * NC + c
                            mm = nc.tensor.matmul(
                                ps_s[u % NPSUM][G * jj:G * (jj + 1), :cn],
                                selB_s[:],
                                stream(sqb_s[(J // 2) % NSQB], CB * G, cn,
                                       (J % 2) * SF + co),
                                start=False, stop=True, skip_group_check=True,
                                tile_position=(0, G * jj))
                            if jj == L - 1 and c == NC - 1:
                                mm.then_inc(sP, 1)

        @blk.scalar
        def _(ac):
            ac.wait_ge(sG, 1)
            for s in range(NSLAB):
                for b in range(NB):
                    gb = s * NB + b
                    L = len(cfg.batches[b])
                    gJ0 = s * NS + b * SBATCH
                    for k in range(L // 2):
                        gpi = gJ0 // 2 + k
                        for J in (2 * gpi, 2 * gpi + 1):
                            if cfg.b_dve[J]:
                                ac.wait_ge(sV, cfg.cnt_v[J])
                            else:
                                ac.wait_ge(sPl, cfg.cnt_p[J])
                        if gpi - NSQB >= 0:
                            ac.wait_ge(
                                sP, cfg.slot_batch(2 * (gpi - NSQB) + 1) + 1)
                        nc.scalar.activation(
                            stream(sqb_s[gpi % NSQB], CB * G, 2 * SF),
                            stream(db_s[gpi % NDP], CB * G, 2 * SF),
                            Act.Square).then_inc(sAq, 1)
                    pb = G * L
                    ac.wait_ge(sP, gb + 1)
                    for c, (co, cn) in enumerate(cfg.chunks):
                        u = gb * NC + c
                        nc.scalar.activation(
                            kt_s[u % 2][:pb, :cn], ps_s[u % NPSUM][:pb, :cn],
                            Act.Exp, scale=EXP_SCALE,
                            accum_out=acc_s[:pb, u:u + 1]).then_inc(sA, 1)
    return nc


# ---------------- host side ----------------

def _block_q(plane, cfg):
    """[H, W] -> flat blocked+haloed [G*Hp*WBH] fp16, zero-padded borders."""
    p = np.zeros((cfg.Hp, cfg.W + 2 * R), dtype=np.float32)
    p[R:R + cfg.H, R:R + cfg.W] = plane
    out = np.empty((G, cfg.Hp, cfg.WBH), dtype=np.float16)
    for g in range(G):
        out[g] = p[:, g * cfg.WB:g * cfg.WB + cfg.WBH]
    return np.ascontiguousarray(out).reshape(-1)


def host_precompute(rgb, depth, depth_gt, depth_mask, depth_gt_mask,
                    xy1_grid, Ts, cfg, b):
    tb = b ^ 1
    xy1 = np.asarray(xy1_grid[b], np.float32)
    dep = np.asarray(depth[b, 0], np.float32)
    dgt_b = np.asarray(depth_gt[b, 0], np.float32)
    dgt_t = np.asarray(depth_gt[tb, 0], np.float32)
    mp = np.asarray(depth_mask[b, 0], np.float32)
    mg_b = np.asarray(depth_gt_mask[b, 0], np.float32)
    mg_t = np.asarray(depth_gt_mask[tb, 0], np.float32)

    xyz_p = xy1 * dep
    T21 = (np.linalg.inv(np.asarray(Ts[tb], np.float64)) @
           np.asarray(Ts[b], np.float64)).astype(np.float32)
    Rm, tv = T21[:3, :3], T21[:3, 3]
    txyz = np.einsum('ij,jhw->ihw', Rm, xyz_p).astype(np.float32) \
        + tv[:, None, None].astype(np.float32)
    pos = (txyz[2] > 0).astype(np.float32) * mp

    qa = np.empty((2, CA, cfg.NQ), np.float16)
    ra = np.empty((2, CA, cfg.NQ), np.float16)
    for c in range(3):
        qa[0, c] = _block_q(xyz_p[c], cfg)
        qa[1, c] = _block_q(txyz[c], cfg)
        ra[0, c] = _block_q(xy1[c] * dgt_b, cfg)
        ra[1, c] = _block_q(xy1[c] * dgt_t, cfg)
    # mask channel: (ra3 - qa3)^2 = 0 iff both masks pass, else >= 400
    qa[0, 3] = -MK * (1.0 - _block_q(mp, cfg))
    qa[1, 3] = -MK * (1.0 - _block_q(pos, cfg))
    ra[0, 3] = MK * (1.0 - _block_q(mg_b, cfg))
    ra[1, 3] = MK * (1.0 - _block_q(mg_t, cfg))
    qb = np.stack([_block_q(np.asarray(rgb[b, c], np.float32), cfg)
                   for c in range(3)])
    rbt = np.stack([_block_q(np.asarray(rgb[tb, c], np.float32), cfg)
                    for c in range(3)])
    return {"qa_d": qa, "ra_d": ra, "qb_d": qb, "rbt_d": rbt,
            "selA_d": make_selA(), "selB_d": make_selB()}


def make_in_maps(rgb, depth, depth_gt, depth_mask, depth_gt_mask, xy1_grid, Ts,
                 cfg, n_cores=8):
    return [host_precompute(rgb, depth, depth_gt, depth_mask, depth_gt_mask,
                            xy1_grid, Ts, cfg, b) for b in range(n_cores)]


_CACHED = {}


def _get_nc(cfg_key=(352, 1216, 32)):
    if cfg_key not in _CACHED:
        cfg = Cfg(*cfg_key)
        nc = bass.Bass()
        emit(nc, cfg)
        _CACHED[cfg_key] = (nc, cfg)
    return _CACHED[cfg_key]


def kernel(rgb, depth, depth_gt, depth_mask, depth_gt_mask, xy1_grid, Ts,
           **run_kwargs):
    from concourse.bass_utils import run_bass_kernel_spmd
    nc, cfg = _get_nc()
    maps = make_in_maps(rgb, depth, depth_gt, depth_mask, depth_gt_mask,
                        xy1_grid, Ts, cfg)
    res = run_bass_kernel_spmd(nc, maps, list(range(8)), **run_kwargs)
    total = np.float64(0.0)
    for r in res.results:
        total += np.float64(r["out_d"][:, 0].sum())
    n_gt = max(np.asarray(depth_gt_mask, np.float64).sum(), 1.0)
    loss = -total / n_gt
    kernel.last_results = res
    return np.float32(loss)
